# revision 11
# baseline (speedup 1.0000x reference)
"""Trainium2 Bass kernel for the ensembler vote-histogram problem.

Computation (reference):
    pred = argmax(expert_logits, axis=-1)            # [E, B, S]
    w    = 1 + noise * 0.001                         # [E, B, S]
    out[b,s,c] = sum_e w[e,b,s] * (pred[e,b,s] == c) # [B, S, C]

Shapes: expert_logits [10, 128, 4000, 5] f32, noise [10, 128, 4000] f32,
out [128, 4000, 5] f32.

Strategy (8 NeuronCores, data-parallel over the batch dim, 16 batches/core):

Per core (64000 tokens, 10 experts, 5 classes):
- SBUF partition layout: 120 partitions = (group g in 0..11) x (expert e in
  0..9), p = 10*g + e.  Each row holds a contiguous run of TAU=5334 tokens
  (group 11 overlaps group 10 by 8 tokens so 12*5334 covers 64000 exactly;
  the overlap is computed twice and stored once).
- VectorE: in-place prefix-max over the 5 classes (4 strided max ops), then
  PM_c = (max <= prefix_max_c) -- a 0/1 step function that rises at the
  FIRST argmax (tie-exact), then nPM_c = 0.001 * noise * PM_c.  PM/nPM are
  written in bf16 into a 6-slot-per-token "gap" layout whose slot 0 is
  always zero.
- TensorE: votes = sum_e (PM_c - PM_{c-1}) + sum_e (nPM_c - nPM_{c-1}),
  computed as 4 accumulating matmuls per PSUM sub-tile with a constant
  block lhs (+1 / -1 per group); the c-1 shift is just the gap view offset
  by one slot, with slot 0 providing the PM_{-1} = 0 column.
- ScalarE copies PSUM -> SBUF; DMA stores contiguous token-major output.
"""
import sys

sys.path.insert(0, "/opt/trn_rl_repo")

import numpy as np

E, B, S, C = 10, 128, 4000, 5
NCORES = 8
BL = B // NCORES            # 16 batches per core
NTOK = BL * S               # 64000 tokens per core
G = 12                      # token groups -> 120 partitions
P = G * E
TAU = 5334                  # tokens per group row
GS = [g * TAU for g in range(G - 1)] + [NTOK - TAU]   # group start tokens
ROW11_SKIP = GS[G - 2] + TAU - GS[G - 1]              # 8 overlap tokens
TC = 1632                   # DVE chunk tokens (multiple of 408)
SBUF_BUFS = 2               # x/n/pm/npm buffering depth
PSUM_BUFS = 2
STAGE_BUFS = 3
SUBT = 102                  # tokens per matmul (510 columns <= 512, 1 bank)
BANKW = 512                 # f32 elements per PSUM bank
PSUB = 4                    # sub-tiles (banks) per PSUM tile -> 408 tokens

_CACHE = {}


def _build():
    import concourse.bacc as bacc
    import concourse.mybir as mybir
    import concourse.tile as tile

    nc = bacc.Bacc("TRN2", target_bir_lowering=False, debug=False,
                   num_devices=NCORES)
    xd = nc.dram_tensor("expert_logits", (E, BL, S, C), mybir.dt.float32,
                        kind="ExternalInput").ap()
    nd = nc.dram_tensor("noise", (E, BL, S), mybir.dt.float32,
                        kind="ExternalInput").ap()
    od = nc.dram_tensor("out", (BL, S, C), mybir.dt.float32,
                        kind="ExternalOutput").ap()

    with tile.TileContext(nc) as tc:
        _kernel(tc, od, xd, nd, variant="fine")
    nc.compile()
    return nc


def _kernel(tc, od, xd, nd, variant="full"):
    import concourse.mybir as mybir

    nc = tc.nc
    op = mybir.AluOpType
    xf = xd.rearrange("e b s c -> e (b s c)")            # [10, 320000]
    nf = nd.rearrange("e b s -> e (b s)")                # [10, 64000]
    of = od.rearrange("b s c -> (b s c)").unsqueeze(0)   # [1, 320000]

    # DRAM views for the uniform groups 0..10 (partition-majorised (g e))
    xa = xf[:, :(G - 1) * TAU * C].rearrange("e (g w) -> e g w", g=G - 1) \
        .transpose([1, 0, 2])                            # [11, 10, TAU*C]
    na = nf[:, :(G - 1) * TAU].rearrange("e (g w) -> e g w", g=G - 1) \
        .transpose([1, 0, 2])                            # [11, 10, TAU]
    # group 11 rows
    xb = xf[:, GS[G - 1] * C:GS[G - 1] * C + TAU * C]
    nb = nf[:, GS[G - 1]:GS[G - 1] + TAU]
    # output rows for groups 0..10
    oa = of[:, :(G - 1) * TAU * C].rearrange("o (g w) -> (o g) w", g=G - 1)

    with tc.tile_pool(name="const", bufs=1) as cpool, \
         tc.tile_pool(name="sbuf", bufs=SBUF_BUFS) as pool, \
         tc.tile_pool(name="psum", bufs=PSUM_BUFS, space="PSUM") as ppool, \
         tc.tile_pool(name="stage", bufs=STAGE_BUFS) as spool:
        # Constant lhsT tiles: lp[p, m] = (p // 10 == m), lm = -lp,
        # built from iota v[p, m] = 10*m - p   (p//10 == m  <=>  -9 <= v <= 0)
        lp = cpool.tile([P, G], mybir.dt.bfloat16)
        lm = cpool.tile([P, G], mybir.dt.bfloat16)
        vio = cpool.tile([P, G], mybir.dt.float32)
        va = cpool.tile([P, G], mybir.dt.float32)
        nc.gpsimd.iota(vio[:], pattern=[[10, G]], base=0, channel_multiplier=-1,
                       allow_small_or_imprecise_dtypes=True)
        nc.vector.tensor_scalar(out=va[:], in0=vio[:], scalar1=-9.0,
                                scalar2=None, op0=op.is_ge)
        nc.vector.scalar_tensor_tensor(out=lp[:], in0=vio[:], scalar=0.0,
                                       in1=va[:], op0=op.is_le, op1=op.mult)
        nc.vector.tensor_scalar(out=lm[:], in0=lp[:], scalar1=-1.0,
                                scalar2=None, op0=op.mult)

        for t0 in range(0, TAU, TC):
            tcl = min(TC, TAU - t0)
            xt = pool.tile([P, TC * C], mybir.dt.float32, tag="x")
            nt = pool.tile([P, TC], mybir.dt.float32, tag="n")
            pm = pool.tile([P, TC * (C + 1)], mybir.dt.bfloat16, tag="pm")
            npm = pool.tile([P, TC * (C + 1)], mybir.dt.bfloat16, tag="npm")

            # loads
            nc.sync.dma_start(out=xt[:(G - 1) * E, :tcl * C],
                              in_=xa[:, :, t0 * C:(t0 + tcl) * C])
            nc.sync.dma_start(out=xt[(G - 1) * E:P, :tcl * C],
                              in_=xb[:, t0 * C:(t0 + tcl) * C])
            nc.sync.dma_start(out=nt[:(G - 1) * E, :tcl],
                              in_=na[:, :, t0:t0 + tcl])
            nc.sync.dma_start(out=nt[(G - 1) * E:P, :tcl],
                              in_=nb[:, t0:t0 + tcl])

            if variant == "dmaonly":
                continue

            # views over the whole chunk
            xv = xt[:, :tcl * C].rearrange("p (t c) -> p t c", c=C)
            pmg = pm[:, :tcl * (C + 1)].rearrange("p (t s) -> p t s", s=C + 1)
            npmg = npm[:, :tcl * (C + 1)].rearrange("p (t s) -> p t s", s=C + 1)
            nc.gpsimd.memset(pmg[:, :, 0], 0.0)
            nc.gpsimd.memset(npmg[:, :, 0], 0.0)

            def dve_piece(lo, hi):
                # in-place prefix max along classes: x becomes Q
                for c in range(1, C):
                    nc.vector.tensor_max(out=xv[:, lo:hi, c],
                                         in0=xv[:, lo:hi, c],
                                         in1=xv[:, lo:hi, c - 1])
                m_b = xv[:, lo:hi, C - 1:C].broadcast_to((P, hi - lo, C))
                nc.vector.tensor_tensor(out=pmg[:, lo:hi, 1:C + 1], in0=m_b,
                                        in1=xv[:, lo:hi, :], op=op.is_le)
                n_b = nt[:, lo:hi].unsqueeze(2).broadcast_to((P, hi - lo, C))
                if variant != "nonpm":
                    nc.vector.scalar_tensor_tensor(
                        out=npmg[:, lo:hi, 1:C + 1],
                        in0=pmg[:, lo:hi, 1:C + 1], scalar=0.001, in1=n_b,
                        op0=op.mult, op1=op.mult)

            if variant != "fine":
                dve_piece(0, tcl)

            # PE + PSUM->SBUF + store, per PSUM tile of up to PSUB banks
            tt0 = 0
            while tt0 < tcl:
                nsub = min(PSUB, -(-(tcl - tt0) // SUBT))
                tok_here = min(PSUB * SUBT, tcl - tt0)
                if variant == "fine":
                    dve_piece(tt0, tt0 + tok_here)
                ps = ppool.tile([G, PSUB * BANKW], mybir.dt.float32, tag="ps")
                subs = []
                for j in range(nsub):
                    tj = tt0 + j * SUBT
                    tl = min(SUBT, tcl - tj)
                    subs.append((j, tj, tl))
                # order matmuls to minimise weight swaps: lp streams then lm
                for sign, lhs, ofs in ((0, lp, 1), (1, lm, 0)):
                    for k, (j, tj, tl) in enumerate(subs):
                        full = pmg[:, tj:tj + tl, ofs:ofs + C]
                        nfull = npmg[:, tj:tj + tl, ofs:ofs + C]
                        is_first = (sign == 0)
                        is_last = (sign == 1)
                        if variant == "nonpm":
                            nc.tensor.matmul(
                                out=ps[:, j * BANKW:j * BANKW + tl * C],
                                lhsT=lhs[:], rhs=full,
                                start=is_first, stop=is_last,
                                skip_group_check=True)
                        else:
                            nc.tensor.matmul(
                                out=ps[:, j * BANKW:j * BANKW + tl * C],
                                lhsT=lhs[:], rhs=full,
                                start=is_first, stop=False,
                                skip_group_check=True)
                            nc.tensor.matmul(
                                out=ps[:, j * BANKW:j * BANKW + tl * C],
                                lhsT=lhs[:], rhs=nfull,
                                start=False, stop=is_last,
                                skip_group_check=True)

                # PSUM -> SBUF (ScalarE), one op per PSUM tile
                st = spool.tile([G, PSUB * SUBT * C], mybir.dt.float32,
                                tag="st")
                for j, tj, tl in subs:
                    nc.scalar.copy(out=st[:, j * SUBT * C:j * SUBT * C + tl * C],
                                   in_=ps[:, j * BANKW:j * BANKW + tl * C])

                # stores: groups 0..10 in one DMA; group 11 separately
                glo = t0 + tt0                     # group-local token start
                ghi = glo + tok_here
                nc.sync.dma_start(out=oa[:, glo * C:ghi * C],
                                  in_=st[:G - 1, :tok_here * C])
                if glo >= ROW11_SKIP:
                    nc.sync.dma_start(
                        out=of[:, (GS[G - 1] + glo) * C:(GS[G - 1] + ghi) * C],
                        in_=st[G - 1:G, :tok_here * C])
                else:
                    skip = ROW11_SKIP - glo
                    nc.sync.dma_start(
                        out=of[:, (GS[G - 1] + ROW11_SKIP) * C:
                               (GS[G - 1] + ghi) * C],
                        in_=st[G - 1:G, skip * C:tok_here * C])
                tt0 += tok_here


def _build_loop(reps, variant="full"):
    """Benchmark variant: the whole kernel body repeated `reps` times inside
    an on-device For_i loop, so device time dominates host dispatch."""
    import concourse.bacc as bacc
    import concourse.mybir as mybir
    import concourse.tile as tile

    nc = bacc.Bacc("TRN2", target_bir_lowering=False, debug=False,
                   num_devices=NCORES)
    xd = nc.dram_tensor("expert_logits", (E, BL, S, C), mybir.dt.float32,
                        kind="ExternalInput").ap()
    nd = nc.dram_tensor("noise", (E, BL, S), mybir.dt.float32,
                        kind="ExternalInput").ap()
    od = nc.dram_tensor("out", (BL, S, C), mybir.dt.float32,
                        kind="ExternalOutput").ap()
    with tile.TileContext(nc) as tc:
        with tc.For_i(0, reps, 1,
                      hint_engines=(mybir.EngineType.PE,
                                    mybir.EngineType.SP)):
            _kernel(tc, od, xd, nd, variant=variant)
    nc.compile()
    return nc


def _get_nc():
    if "nc" not in _CACHE:
        _CACHE["nc"] = _build()
    return _CACHE["nc"]


def _run(inputs, trace=False):
    from concourse import bass_utils

    nc = _get_nc()
    x = np.ascontiguousarray(inputs["expert_logits"], dtype=np.float32)
    n = np.ascontiguousarray(inputs["noise"], dtype=np.float32)
    in_maps = []
    for k in range(NCORES):
        bsl = slice(k * BL, (k + 1) * BL)
        in_maps.append({
            "expert_logits": np.ascontiguousarray(x[:, bsl]),
            "noise": np.ascontiguousarray(n[:, bsl]),
        })
    res = bass_utils.run_bass_kernel_spmd(
        nc, in_maps, core_ids=list(range(NCORES)), trace=trace)
    out = np.concatenate([r["out"] for r in res.results], axis=0)
    return out, res


def kernel(**inputs) -> np.ndarray:
    out, _ = _run(inputs, trace=False)
    return out
